# revision 24
# baseline (speedup 1.0000x reference)
"""Trainium2 Bass kernel for nn_DIAGCN (RGCN + GraphConv + classifier over
block-diagonal dialog graphs), SPMD over 8 NeuronCores.

Strategy (v5)
-------------
The dialog graph is a causal 5-tap window (edges i -> i+o, o = 0..4, within
each 100-utterance dialog), and relation_type(i,j) = spk[i]*spk[j] with spk
derived from self-edges.  Every per-node linear map commutes with both the
window sum W(.) (row-mixing) and per-node diagonal scalings (row scaling), so
the whole network folds into 7-wide channels:

    out = W(g0) + f0
    g0  = x@(w_root@wA) + cA + ic0.*W(a0) - ic0s.*W(spk.*a0) + ic1s.*W(spk.*a1)
    f0  = x@(w_root@wB + w_skip@w_clf) + cBc + ic0.*W(b0) - ic0s.*W(spk.*b0)
          + ic1s.*W(spk.*b1)
    a{0,1} = x@(w_rel{0,1}@wA), b{0,1} = x@(w_rel{0,1}@wB)
    wA  = w_gc_rel@w_clf, wB = w_gc_root@w_clf

Device pipeline per 512-column tile (everything bf16, f32 PSUM):
  1. x^T comes in as 2-MiB two-tile DMAs (16 KiB contiguous per partition;
     few, fat transfers keep the HWDGE ring shallow and all 16 SDMA engines
     evenly loaded),
  2. 8 accumulating matmuls: ps[80, 512] = Wbig^T @ x^T,
  3. DVE forms the spk-scaled window inputs (28 rows), ACT copies the plain
     rows and the raw channels,
  4. window tree: T1 (Pool, odd shift), T2/WT (DVE, even shifts), then one
     DVE multiply applies the per-node mean coefficients -> V,
  5. the outer window W(g0) is folded into the channel-reduction matmuls via
     W5 = (I+sh1)(I+sh2) + sh4:  V2 = V + sh2(V) (Pool), then
     ps2 = S_A^T @ (V2 + sh1 V2 + sh4 V) + S_B^T @ V  (4 matmuls),
  6. one ACT copy ps2 -> out rows; output DMAed in 5 chunks.

Nodes are sharded by dialog (no cross-core edges), 64 padded dialogs per
core; each dialog is 4 zero "gap" columns + 100 data columns so neither
window leaks across dialogs.  A 4-column zero guard at the left edge of the
column grid removes all tile-0 special cases.  Side tensors are split across
SBUF partition parity (coef at partitions 0.., spk at 64..) so their DMA
descriptors spread over both even and odd SDMA engines.
"""
import numpy as np
import ml_dtypes

# ---------------------------------------------------------------- constants
B, L, FUT = 500, 100, 4
N = B * L
IN, HID, NCLS = 1024, 512, 7
NCORES = 8
GAP = 4
DLG = L + GAP            # 104 columns per dialog
DPC = 64                 # padded dialogs per core
COLS = DPC * DLG         # 6656 columns per core
GC = COLS + 4            # guarded column grid (4 zero cols at the left)
NT = 13                  # column tiles
NTC = COLS // NT         # 512
KB = IN // 128           # 8 contraction blocks
M = 80                   # Wbig columns (psum partitions)
WR = 46                  # windowed rows 0..45
NSP = 32                 # spk rows 0..27 + zero rows 28..31
WSS = KB * M + 16        # packed const free dim (Wbig + S_A + S_B)
# column chunks for the side-tensor DMAs (guarded grid): each lands in the
# sync FIFO just ahead of the xt tiles that consume it
SIDE_CH = [(0, 2052), (2052, 4100), (4100, 6148), (6148, 6660)]

D_COUNTS = [63, 63, 63, 63, 62, 62, 62, 62]
D_STARTS = np.concatenate([[0], np.cumsum(D_COUNTS)])[:-1]

# Wbig / V row map
R_A0S, R_A1S, R_B0S, R_B1S = 0, 7, 14, 21
R_A0, R_B0 = 32, 39
R_RA, R_FSC = 64, 71
R_VMASK = 48             # V row: plain real-node mask (bias channel)
R_SPK = 64               # spk rows 64..91 of the aux tile (odd SDMA engines)

BF16 = ml_dtypes.bfloat16


def _data_cols():
    d = np.arange(DPC)[:, None]
    u = np.arange(L)[None, :]
    return d * DLG + GAP + u  # [DPC, L]


# ---------------------------------------------------------------- host prep
def _check_graph(edges, relation_type):
    i = np.arange(L)[:, None]
    off = np.arange(FUT + 1)[None, :]
    tl = i + off
    valid = tl < L
    sl = np.broadcast_to(i, tl.shape)[valid]
    tl = tl[valid]
    base = (np.arange(B) * L)[:, None]
    src = (base + sl[None, :]).reshape(-1)
    tgt = (base + tl[None, :]).reshape(-1)
    if edges.shape != (2, src.size) or not (
        np.array_equal(edges[0], src) and np.array_equal(edges[1], tgt)
    ):
        raise ValueError("edge structure does not match the DIAGCN pattern")
    sel = edges[0] == edges[1]
    spk = np.zeros(N, dtype=np.float64)
    spk[edges[0][sel]] = relation_type[sel]
    return spk


def _host_prep(x, edges, relation_type, w_rel, w_root, b_rgcn,
               w_gc_rel, w_gc_root, b_gc, w_skip, b_skip, w_clf, b_clf):
    x = np.asarray(x, dtype=np.float32)
    edges = np.asarray(edges)
    relation_type = np.asarray(relation_type)
    spk = _check_graph(edges, relation_type)

    tgt = edges[1]
    c1 = np.bincount(tgt[relation_type == 1], minlength=N).astype(np.float64)
    c0 = np.bincount(tgt[relation_type == 0], minlength=N).astype(np.float64)
    ic0 = 1.0 / np.maximum(c0, 1.0)
    ic1 = 1.0 / np.maximum(c1, 1.0)
    ic0s = ic0 * spk
    ic1s = ic1 * spk

    f8 = lambda a: np.asarray(a, dtype=np.float64)
    w_rel, w_root, w_gc_rel, w_gc_root, w_skip, w_clf = map(
        f8, (w_rel, w_root, w_gc_rel, w_gc_root, w_skip, w_clf))
    b_rgcn, b_gc, b_skip, b_clf = map(f8, (b_rgcn, b_gc, b_skip, b_clf))

    wA = w_gc_rel @ w_clf
    wB = w_gc_root @ w_clf
    Wbig = np.zeros((IN, M), dtype=np.float64)
    Wbig[:, R_A0S:R_A0S + 7] = w_rel[0] @ wA
    Wbig[:, R_A1S:R_A1S + 7] = w_rel[1] @ wA
    Wbig[:, R_B0S:R_B0S + 7] = w_rel[0] @ wB
    Wbig[:, R_B1S:R_B1S + 7] = w_rel[1] @ wB
    Wbig[:, R_A0:R_A0 + 7] = w_rel[0] @ wA
    Wbig[:, R_B0:R_B0 + 7] = w_rel[0] @ wB
    Wbig[:, R_RA:R_RA + 7] = w_root @ wA
    Wbig[:, R_FSC:R_FSC + 7] = w_root @ wB + w_skip @ w_clf

    cA = b_rgcn @ wA
    cBc = b_rgcn @ wB + (b_gc + b_skip) @ w_clf + b_clf
    SA = np.zeros((M, NCLS), dtype=np.float64)
    SB = np.zeros((M, NCLS), dtype=np.float64)
    for i in range(7):
        SA[R_A0S + i, i] = 1.0
        SA[R_A1S + i, i] = 1.0
        SA[R_A0 + i, i] = 1.0
        SA[R_RA + i, i] = 1.0
        SB[R_B0S + i, i] = 1.0
        SB[R_B1S + i, i] = 1.0
        SB[R_B0 + i, i] = 1.0
        SB[R_FSC + i, i] = 1.0
    SA[R_VMASK] = cA
    SB[R_VMASK] = cBc

    # packed consts: [128, KB*M] Wbig (partition p holds rows {k*128+p}),
    # then S_A at cols KB*M..+7, S_B at +7..+14 (partitions 0..79)
    wss = np.zeros((128, WSS), dtype=np.float64)
    wss[:, :KB * M] = Wbig.reshape(KB, 128, M).swapaxes(0, 1).reshape(128, KB * M)
    wss[0:M, KB * M:KB * M + 7] = SA
    wss[0:M, KB * M + 7:KB * M + 14] = SB
    wss = wss.astype(BF16)

    dc = _data_cols()
    in_maps = []
    unshard_info = []
    for c in range(NCORES):
        nd = D_COUNTS[c]
        g0 = D_STARTS[c]
        cols_real = dc[:nd].reshape(-1)
        nodes_real = g0 * L + np.arange(nd * L)

        xt_full = np.zeros((IN, COLS), dtype=np.float32)
        xt_full[:, cols_real] = x[nodes_real].T
        # partition p, kblock k <- feature row k*128+p
        xt = xt_full.reshape(KB, 128, COLS).swapaxes(0, 1)
        # [NT, 128, KB*NTC]: per partition 8 KiB contiguous per tile
        xts = np.ascontiguousarray(
            xt.reshape(128, KB, NT, NTC).transpose(2, 0, 1, 3)
            .reshape(NT, 128, KB * NTC)).astype(BF16)

        def vec_to_cols(v):
            out = np.zeros(GC, dtype=np.float64)
            out[4 + cols_real] = v[nodes_real]
            return out

        spk_c = vec_to_cols(spk)
        ic0_c = vec_to_cols(ic0)
        ic0s_c = vec_to_cols(ic0s)
        ic1s_c = vec_to_cols(ic1s)
        mask_c = np.zeros(GC, dtype=np.float64)
        mask_c[4 + cols_real] = 1.0

        spk32 = np.zeros((NSP, GC), dtype=np.float64)
        spk32[0:28] = spk_c
        coefr = np.zeros((WR, GC), dtype=np.float64)
        coefr[R_A0S:R_A0S + 7] = -ic0s_c
        coefr[R_A1S:R_A1S + 7] = ic1s_c
        coefr[R_B0S:R_B0S + 7] = -ic0s_c
        coefr[R_B1S:R_B1S + 7] = ic1s_c
        coefr[R_A0:R_A0 + 7] = ic0_c
        coefr[R_B0:R_B0 + 7] = ic0_c

        in_maps.append(dict(
            xt=xts, wss=wss,
            spk32=spk32.astype(BF16), coefr=coefr.astype(BF16),
            vmask=mask_c[None, :].astype(BF16),
        ))
        unshard_info.append((nodes_real, cols_real))
    return in_maps, unshard_info


# ---------------------------------------------------------------- bass kernel
_COMPILED = None


def _build():
    import concourse.bass as bass
    from concourse import bacc
    import concourse.mybir as mybir
    from concourse.tile import TileContext

    f32 = mybir.dt.float32
    bf16 = mybir.dt.bfloat16
    ADD = mybir.AluOpType.add
    MUL = mybir.AluOpType.mult

    nc = bacc.Bacc("TRN2", target_bir_lowering=False, debug=False,
                   num_devices=NCORES)
    xt_d = nc.dram_tensor("xt", [NT, 128, KB * NTC], bf16,
                          kind="ExternalInput")
    wss_d = nc.dram_tensor("wss", [128, WSS], bf16, kind="ExternalInput")
    spk_d = nc.dram_tensor("spk32", [NSP, GC], bf16, kind="ExternalInput")
    coefr_d = nc.dram_tensor("coefr", [WR, GC], bf16, kind="ExternalInput")
    vmask_d = nc.dram_tensor("vmask", [1, GC], bf16, kind="ExternalInput")
    y_d = nc.dram_tensor("y", [NCLS, COLS], bf16, kind="ExternalOutput")

    with TileContext(nc) as tc:
        with (
            tc.tile_pool(name="const", bufs=1) as cpool,
            tc.tile_pool(name="xin", bufs=6) as xpool,
            tc.tile_pool(name="wrk", bufs=3) as wpool,
            tc.tile_pool(name="psum", bufs=4, space="PSUM") as ppool,
            tc.tile_pool(name="psum2", bufs=4, space="PSUM") as p2pool,
        ):
            tCF = cpool.tile([WR, GC], bf16)    # coefficients (even engines)
            tAux = cpool.tile([128, GC], bf16)  # spk rows 64..95 (odd engines)
            tZ = cpool.tile([WR, GC], bf16)     # windowed inputs
            tT1 = cpool.tile([WR, GC], bf16)    # first window stage
            tV = cpool.tile([128, GC], bf16)    # S-matmul moving operand
            tV2 = cpool.tile([128, GC], bf16)   # V + sh2(V)
            tOut = cpool.tile([NCLS, COLS], bf16)
            wss = cpool.tile([128, WSS], bf16)

            # rows 46..63, 78..95 of V must be non-NaN for the S matmuls
            # (their S rows are zero); rows 64..77 rewritten per tile.
            # engine ops span <=32 partitions from a non-zero base -> 2 ops
            nc.gpsimd.memset(tV[32:64, :], 0.0)
            nc.gpsimd.memset(tV[64:96, :], 0.0)
            # guard columns only; Pool memsets are ~60 ns
            nc.gpsimd.memset(tV[0:32, 0:4], 0.0)
            nc.gpsimd.memset(tZ[:, 0:4], 0.0)
            nc.gpsimd.memset(tT1[:, 0:4], 0.0)
            nc.gpsimd.memset(tV2[:, 0:4], 0.0)

            # vmask waits on the tV memset -> keep it off the sync queue
            nc.scalar.dma_start(tV[R_VMASK:R_VMASK + 1], vmask_d[:])

            spk = tAux[R_SPK:R_SPK + NSP]
            KM = KB * M
            ssA = wss[0:M, KM:KM + 7]
            ssB = wss[0:M, KM + 7:KM + 14]

            # sync FIFO: packed consts, then per-chunk side slices woven just
            # ahead of the xt tiles that consume them (nothing here has
            # trigger-time waits, so the queue never head-of-line blocks)
            nc.sync.dma_start(wss[:], wss_d[:])
            xt_tiles = {}

            def stage_chunk(ch):
                lo, hi = SIDE_CH[ch]
                nc.sync.dma_start(tCF[:, lo:hi], coefr_d[:, lo:hi])
                nc.sync.dma_start(spk[:, lo:hi], spk_d[:, lo:hi])

            def stage_xt(t):
                xt_t = xpool.tile([128, KB, NTC], bf16, tag="xt")
                nc.sync.dma_start(xt_t[:], xt_d[t])
                xt_tiles[t] = xt_t

            stage_chunk(0)
            stage_xt(0)
            stage_xt(1)
            stage_xt(2)
            stage_chunk(1)
            stage_xt(3)
            stage_xt(4)
            stage_xt(5)
            stage_chunk(2)
            for t in range(6, 9):
                stage_xt(t)
            stage_chunk(3)
            for t in range(9, NT):
                stage_xt(t)

            for t in range(NT):
                c0 = t * NTC
                g0 = 4 + c0
                cs = slice(g0, g0 + NTC)
                xt_t = xt_tiles[t]
                ps = ppool.tile([M, NTC], f32)
                for k in range(KB):
                    nc.tensor.matmul(
                        ps[:], wss[:, k * M:(k + 1) * M], xt_t[:, k, :],
                        start=(k == 0), stop=(k == KB - 1))
                # windowed inputs: rows 0..27 spk-scaled, 28..31 zeroed by
                # the zero spk rows (DVE); rows 32..45 plain copy (ACT;
                # pad/gap cols of ps are 0)
                nc.vector.tensor_tensor(
                    tZ[0:NSP, cs], ps[0:NSP], spk[:, cs], MUL)
                nc.scalar.copy(tZ[R_A0:R_B0 + 7, cs], ps[R_A0:R_B0 + 7])
                # raw channels (root/skip projections) straight into V
                nc.scalar.copy(tV[R_RA:R_FSC + 7, cs], ps[R_RA:R_FSC + 7])
                # 5-tap causal window as a shift tree:
                #   t1 = z + sh1(z); t2 = t1 + sh2(t1); wt = t2 + sh4(z)
                # odd shift can't use the DVE 16-bit 2x path -> Pool
                nc.gpsimd.tensor_tensor(
                    tT1[:, cs], tZ[:, cs],
                    tZ[:, g0 - 1:g0 + NTC - 1], ADD)
                T2 = wpool.tile([WR, NTC], bf16, tag="T2")
                nc.vector.tensor_tensor(
                    T2[:], tT1[:, cs], tT1[:, g0 - 2:g0 + NTC - 2], ADD)
                WT = wpool.tile([WR, NTC], bf16, tag="WT")
                nc.vector.tensor_tensor(
                    WT[:], T2[:], tZ[:, g0 - 4:g0 + NTC - 4], ADD)
                nc.vector.tensor_tensor(
                    tV[0:WR, cs], WT[:], tCF[:, cs], MUL)
                # W5 = (I+sh1)(I+sh2) + sh4: V2 = V + sh2(V) on Pool
                nc.gpsimd.tensor_tensor(
                    tV2[0:M, cs], tV[0:M, cs],
                    tV[0:M, g0 - 2:g0 + NTC - 2], ADD)
                ps2 = p2pool.tile([NCLS, NTC], f32)
                nc.tensor.matmul(ps2[:], ssA, tV2[0:M, cs],
                                 start=True, stop=False)
                nc.tensor.matmul(ps2[:], ssA,
                                 tV2[0:M, g0 - 1:g0 + NTC - 1],
                                 start=False, stop=False)
                nc.tensor.matmul(ps2[:], ssA,
                                 tV[0:M, g0 - 4:g0 + NTC - 4],
                                 start=False, stop=False)
                nc.tensor.matmul(ps2[:], ssB, tV[0:M, cs],
                                 start=False, stop=True)
                nc.scalar.copy(tOut[:, c0:c0 + NTC], ps2[:])
                if t in (3, 6, 9, 11, 12):
                    lo = {3: 0, 6: 2048, 9: 3584, 11: 5120, 12: 6144}[t]
                    nc.scalar.dma_start(y_d[:, lo:c0 + NTC],
                                        tOut[:, lo:c0 + NTC])
    nc.compile()
    return nc


def _get_compiled():
    global _COMPILED
    if _COMPILED is None:
        _COMPILED = _build()
    return _COMPILED


def _run(in_maps, trace=False):
    from concourse.bass_utils import run_bass_kernel_spmd
    nc = _get_compiled()
    return run_bass_kernel_spmd(nc, in_maps, list(range(NCORES)), trace=trace)


def kernel(**inputs) -> np.ndarray:
    in_maps, unshard_info = _host_prep(**inputs)
    res = _run(in_maps)
    out = np.zeros((N, NCLS), dtype=np.float32)
    for c in range(NCORES):
        nodes_real, cols_real = unshard_info[c]
        out[nodes_real] = res.results[c]["y"][:, cols_real].T.astype(np.float32)
    return out
